# revision 32
# baseline (speedup 1.0000x reference)
"""Distributed Bass kernel for nn_Attention_15247133900834.

Full-input -> full-output multi-head attention block on 8 TRN2 NeuronCores.

Sharding (hardcoded): B=2 batches x 12 heads.  Core i handles batch g=i//4
and heads [3p, 3p+1, 3p+2] with p=i%4.  Each core computes QKV + attention
for its 3 heads over ALL 2048 tokens, then an AllToAll inside each 4-core
batch group exchanges 512-token slices of the per-head attention outputs so
every core ends up with all 768 features for its token slice, and runs the
output projection for those 512 tokens.

Device-side layout choices (host prepares the shards accordingly):
- x is passed TRANSPOSED per batch (x[g].T as [768, 2048] bf16) because every
  matmul contracts over the feature axis, which must live on SBUF partitions.
- Q/K weights are packed per head-pair [wq_hA | wq_hB] so one [768,128] weight
  block yields Q^T of head A on partitions 0-63 and head B on partitions
  64-127; stage-2 S^T = K^T.T @ Q^T then runs as two concurrent row-group
  matmuls (contraction d=64 each) that together use the full 128-row PE array.
  The 3rd (solo) head duplicates its weights so the two row groups process
  low/high key halves concurrently instead.
- Attention probabilities stay transposed (P^T = [keys, q]) so the second
  attention matmul out^T = [V|1].T @ P^T needs no transposes anywhere; the
  extra ones-column of V makes the PE accumulate the softmax denominators
  as psum row 64 for free.  exp() runs on ScalarE straight out of PSUM with
  the attention scale folded into the activation's free affine.
- The task mask (first 4x4 block = eye) is applied as a tiny correction
  matmul that subtracts the masked contributions: rhs = P^T[0:4,0:4]*(eye-1).
  The denominators are computed over the UNMASKED row (mask is applied after
  softmax in the reference).
"""

import numpy as np
import ml_dtypes

import concourse.bass as bass
import concourse.mybir as mybir
import concourse.tile as tile
from concourse import bacc
from concourse.bass_utils import run_bass_kernel_spmd

BF16 = mybir.dt.bfloat16
F32 = mybir.dt.float32
NPBF16 = ml_dtypes.bfloat16

B, N, C = 2, 2048, 768
H, D = 12, 64
NT = 4              # tokens with the eye task-mask
NCORES = 8
GROUP = 4           # cores per batch group
TOKS = N // GROUP   # 512 output tokens per core
HPC = 3             # heads per core
CCH = C // 128      # 6 feature contraction chunks
KCH = N // 128      # 16 key chunks
QT = N // 512       # 4 q tiles
SCALE = D ** -0.5
EXP = mybir.ActivationFunctionType.Exp

REPLICA_GROUPS = [list(range(NCORES))]


def build_graph(dump: bool = False):
    nc = bacc.Bacc(
        "TRN2", target_bir_lowering=False, debug=False, num_devices=NCORES
    )
    dumps = {}

    def dump_tensor(name, shape, dtype):
        if not dump:
            return None
        d = nc.dram_tensor(name, shape, dtype, kind="ExternalOutput")
        dumps[name] = d
        return d
    xt_d = nc.dram_tensor("xt", [C, N], BF16, kind="ExternalInput")
    wqk_d = nc.dram_tensor("wqk", [C, 512], BF16, kind="ExternalInput")
    wv_d = nc.dram_tensor("wv", [C, HPC * D], BF16, kind="ExternalInput")
    wp_d = nc.dram_tensor("wp", [C, C], BF16, kind="ExternalInput")
    bias_d = nc.dram_tensor("bias", [128, C], F32, kind="ExternalInput")
    am_d = nc.dram_tensor("amask", [NT, NT], BF16, kind="ExternalInput")
    out_d = nc.dram_tensor("out", [TOKS, C], F32, kind="ExternalOutput")

    with tile.TileContext(nc) as tc:
        with (
            tc.tile_pool(name="const", bufs=1) as cpool,
            tc.tile_pool(name="work", bufs=3) as wpool,
            tc.tile_pool(name="pt", bufs=4) as ptpool,
            tc.tile_pool(name="ps2", bufs=2, space="PSUM") as ps2,
            tc.tile_pool(name="pso", bufs=4, space="PSUM") as pso,
            tc.tile_pool(name="dram", bufs=1, space="DRAM") as dpool,
        ):
            # ---------------- constant loads ----------------
            xt = cpool.tile([128, CCH, N], BF16, name="xt_sb")
            nc.sync.dma_start(xt, xt_d.ap().rearrange("(c p) n -> p c n", p=128))
            wqk = cpool.tile([128, CCH, 512], BF16, name="wqk_sb")
            nc.sync.dma_start(wqk, wqk_d.ap().rearrange("(c p) m -> p c m", p=128))
            wv = cpool.tile([128, CCH, HPC * D], BF16, name="wv_sb")
            nc.sync.dma_start(wv, wv_d.ap().rearrange("(c p) m -> p c m", p=128))
            wp = cpool.tile([128, CCH, C], BF16, name="wp_sb")
            nc.sync.dma_start(wp, wp_d.ap().rearrange("(c p) m -> p c m", p=128))
            bias = cpool.tile([128, C], F32, name="bias_sb")
            nc.sync.dma_start(bias, bias_d.ap())
            amask = cpool.tile([NT, NT], BF16, name="amask_sb")
            nc.sync.dma_start(amask, am_d.ap())

            # ---------------- stage 1a: Q^T / K^T blocks ----------------
            # qk block m: 0 = [Q_A; Q_B], 1 = [K_A; K_B], 2 = [Q_C; Q_C],
            # 3 = [K_C; K_C]  (head X on partitions 0-63 / 64-127)
            qk = cpool.tile([128, 4, N], BF16, name="qk_sb")

            def make_qk_block(m):
                for t in range(2):
                    pq = ps2.tile(
                        [128, 1024], F32, tag="g", name=f"pq{m}_{t}"
                    )
                    for c in range(CCH):
                        for h in range(2):
                            nc.tensor.matmul(
                                pq[:, h * 512 : (h + 1) * 512],
                                wqk[:, c, m * 128 : (m + 1) * 128],
                                xt[:, c, t * 1024 + h * 512 : t * 1024 + (h + 1) * 512],
                                start=(c == 0),
                                stop=(c == CCH - 1),
                            )
                    nc.vector.tensor_copy(
                        out=qk[:, m, t * 1024 : (t + 1) * 1024], in_=pq
                    )

            # ---------------- stage 1b: V natural + 64 ones columns -------
            # vt[:, k, h*128 : h*128+64] = V_h keys chunk k; cols h*128+64 ..
            # h*128+127 are all-ones, so stage-3's [V_h | 1..1].T @ P^T puts
            # the softmax denominators in psum rows 64-127, already broadcast
            # 64-wide for the normalization multiply.
            vt = cpool.tile([128, KCH, HPC * 128], BF16, name="vt_sb")
            ones_view = vt.rearrange("p k (h e) -> p k h e", e=128)[
                :, :, :, 64:128
            ]
            nc.vector.memset(ones_view, 1.0)

            def make_v_chunk(n_):
                pv = ps2.tile([128, 1024], F32, tag="g", name=f"pv{n_}")
                for c in range(CCH):
                    nc.tensor.matmul(
                        pv[:, 0 : HPC * D],
                        xt[:, c, n_ * 128 : (n_ + 1) * 128],
                        wv[:, c, :],
                        start=(c == 0),
                        stop=(c == CCH - 1),
                    )
                nc.vector.tensor_copy(
                    out=vt.rearrange("p k (h e) -> p k h e", e=128)[
                        :, n_, :, 0:64
                    ],
                    in_=pv[:, 0 : HPC * D].rearrange("p (h e) -> p h e", e=64),
                )

            # Emit only what the pair unit needs up front; the solo head's
            # Q/K blocks are emitted after the pair rounds so they fill PE
            # gaps while ScalarE drains the last pair exps.
            make_qk_block(0)
            make_qk_block(1)
            for n_ in range(KCH):
                make_v_chunk(n_)

            # ---------------- attention rounds ----------------
            attnAB = cpool.tile([128, N], BF16, name="attnAB_sb")
            attnC = cpool.tile([64, N], BF16, name="attnC_sb")

            # AllToAll bounce buffers (declared up front; the heads-A/B
            # exchange is launched right after the pair unit finishes so it
            # overlaps the solo head's compute).  Destination d owns tokens
            # [256d, 256d+256) of BOTH batches; my shard d = my head-features
            # for those tokens.  After the exchange, output shard j = rank
            # j's features for MY tokens: shards 0-3 = batch-0 heads,
            # shards 4-7 = batch-1 heads.
            a2a1_in = dpool.tile([NCORES, 128, 256], BF16, name="a2a1_in")
            a2a1_out = dpool.tile([NCORES, 128, 256], BF16, name="a2a1_out")
            a2a2_in = dpool.tile([NCORES, 64, 256], BF16, name="a2a2_in")
            a2a2_out = dpool.tile([NCORES, 64, 256], BF16, name="a2a2_out")

            # (kind, q block, k block, [(head col base, attn dest fn)])
            units = [
                ("pair", 0, 1),
                ("solo", 2, 3),
            ]
            for kind_, qb, kb in units:
                if kind_ == "pair":
                    heads = [(0 * 128, attnAB, 0), (1 * 128, attnAB, 64)]
                else:
                    heads = [(2 * 128, attnC, 0)]
                for t in range(2):  # two 1024-wide q tiles
                    qs = slice(t * 1024, (t + 1) * 1024)
                    # one psum accumulator per (head, 512-wide q half)
                    pos = [
                        [
                            pso.tile(
                                [128, 512], F32, tag="o",
                                name=f"o{kind_}{t}_{i}_{h}",
                            )
                            for h in range(2)
                        ]
                        for i in range(len(heads))
                    ]
                    started = [[False] * 2 for _ in heads]
                    # pair: both heads see chunks 0..15.  solo: the top
                    # row-group covers chunks 0..7 while the bottom one
                    # concurrently covers 8..15.
                    nch = KCH if kind_ == "pair" else KCH // 2
                    for cc in range(nch):
                        gA = ps2.tile(
                            [128, 1024], F32, tag="g", name=f"gA{kind_}{t}_{cc}"
                        )
                        gB = ps2.tile(
                            [128, 1024], F32, tag="g", name=f"gB{kind_}{t}_{cc}"
                        )
                        # top row-group: head A (pair) / low keys (solo);
                        # bottom row-group: head B (pair) / high keys (solo).
                        # Each 1024-wide q tile is two matmuls (one psum bank
                        # each) sharing the stationary operand.
                        kcol = cc if kind_ == "pair" else 8 + cc
                        for h in range(2):
                            q5h = slice(
                                t * 1024 + h * 512, t * 1024 + (h + 1) * 512
                            )
                            nc.tensor.matmul(
                                gA[:, h * 512 : (h + 1) * 512],
                                qk[0:64, kb, cc * 128 : (cc + 1) * 128],
                                qk[0:64, qb, q5h],
                                start=True,
                                stop=True,
                            )
                            nc.tensor.matmul(
                                gB[:, h * 512 : (h + 1) * 512],
                                qk[64:128, kb, kcol * 128 : (kcol + 1) * 128],
                                qk[64:128, qb, q5h],
                                start=True,
                                stop=True,
                            )
                        ptA = ptpool.tile(
                            [128, 1024], BF16, tag="pt",
                            name=f"ptA{kind_}{t}_{cc}",
                        )
                        ptB = ptpool.tile(
                            [128, 1024], BF16, tag="pt",
                            name=f"ptB{kind_}{t}_{cc}",
                        )
                        nc.scalar.activation(ptA, gA, EXP, scale=SCALE)
                        nc.scalar.activation(ptB, gB, EXP, scale=SCALE)
                        if dump and kind_ == "pair" and t == 0 and cc == 0:
                            nc.sync.dma_start(
                                dump_tensor("dpt", [128, 1024], BF16).ap(),
                                ptA,
                            )

                        # stage 3: out^T += [V_h | 1..1].T @ P^T_chunk
                        if kind_ == "pair":
                            mms = [(0, cc, ptA), (1, cc, ptB)]
                        else:
                            mms = [(0, cc, ptA), (0, 8 + cc, ptB)]
                        for h in range(2):
                            for i, kchunk, pt_ap in mms:
                                colb = heads[i][0]
                                nc.tensor.matmul(
                                    pos[i][h],
                                    vt[:, kchunk, colb : colb + 128],
                                    pt_ap[:, h * 512 : (h + 1) * 512],
                                    start=not started[i][h],
                                    stop=(kchunk == KCH - 1),
                                )
                                started[i][h] = True

                        # task-mask correction (q rows 0-3 x key rows 0-3)
                        if t == 0 and cc == 0:
                            for i in range(len(heads)):
                                colb = heads[i][0]
                                anti = wpool.tile(
                                    [128, NT], BF16, tag="anti",
                                    name=f"anti{kind_}{i}",
                                )
                                nc.vector.memset(anti, 0.0)
                                nc.vector.tensor_mul(
                                    out=anti[0:NT, :],
                                    in0=ptA[0:NT, 0:NT],
                                    in1=amask,
                                )
                                nc.tensor.matmul(
                                    pos[i][0][0:64, 0:NT],
                                    vt[:, 0, colb : colb + 64],
                                    anti,
                                    start=False,
                                    stop=False,
                                )

                    # normalization: psum rows 64-127 already hold the softmax
                    # denominators broadcast 64-wide (the ones columns of vt).
                    # Copy each psum tile to SBUF first so the psum slot frees
                    # quickly; the slow reciprocal + multiply then run off the
                    # psum-pool critical path.
                    for i, (colb, dst, drow) in enumerate(heads):
                        for h in range(2):
                            q5 = slice(
                                t * 1024 + h * 512, t * 1024 + (h + 1) * 512
                            )
                            ocp = wpool.tile(
                                [128, 512], F32, tag="ocp",
                                name=f"oc{kind_}{t}_{i}_{h}",
                            )
                            nc.vector.tensor_copy(out=ocp, in_=pos[i][h])
                            rec = wpool.tile(
                                [64, 512], F32, tag="rec",
                                name=f"rc{kind_}{t}_{i}_{h}",
                            )
                            nc.vector.reciprocal(out=rec, in_=ocp[64:128, :])
                            nc.vector.tensor_mul(
                                out=dst[drow : drow + 64, q5],
                                in0=ocp[0:64, :],
                                in1=rec,
                            )
                            if (
                                dump and kind_ == "pair" and t == 0
                                and i == 0 and h == 0
                            ):
                                nc.sync.dma_start(
                                    dump_tensor("dpos", [128, 512], F32).ap(),
                                    ocp,
                                )
                                nc.sync.dma_start(
                                    dump_tensor("drec", [64, 512], F32).ap(),
                                    rec,
                                )

                # after the pair unit's four q-tiles, attnAB is complete:
                # launch its AllToAll so the exchange overlaps the solo
                # head's compute, and emit the solo head's Q/K production
                # (it fills PE gaps while ScalarE drains the last pair exps).
                if kind_ == "pair":
                    for d in range(NCORES):
                        nc.sync.dma_start(
                            a2a1_in[d], attnAB[:, d * 256 : (d + 1) * 256]
                        )
                    nc.gpsimd.collective_compute(
                        "AllToAll",
                        mybir.AluOpType.bypass,
                        replica_groups=REPLICA_GROUPS,
                        ins=[a2a1_in.opt()],
                        outs=[a2a1_out.opt()],
                    )
                    make_qk_block(2)
                    make_qk_block(3)

            if dump:
                nc.sync.dma_start(
                    dump_tensor("dqk", [128, 4, N], BF16).ap(), qk
                )
                nc.sync.dma_start(
                    dump_tensor("dvt", [128, KCH, HPC * 128], BF16).ap(), vt
                )

            if dump:
                nc.sync.dma_start(
                    dump_tensor("dattnAB", [128, N], BF16).ap(), attnAB
                )
                nc.sync.dma_start(
                    dump_tensor("dattnC", [64, N], BF16).ap(), attnC
                )

            # ---------------- second AllToAll: the solo head --------------
            for d in range(NCORES):
                nc.sync.dma_start(
                    a2a2_in[d], attnC[:, d * 256 : (d + 1) * 256]
                )
            nc.gpsimd.collective_compute(
                "AllToAll",
                mybir.AluOpType.bypass,
                replica_groups=REPLICA_GROUPS,
                ins=[a2a2_in.opt()],
                outs=[a2a2_out.opt()],
            )

            # gather per batch: [768 features, 256 tokens] -> [128, 6, 256].
            # PERMUTED feature order (host permutes w_proj rows to match):
            # chunks 0-3 = the four ranks' A/B heads (from a2a1, available
            # while a2a2 is still in flight), chunks 4-5 = the C heads.
            gaths = []
            for b in range(B):
                gath = cpool.tile([128, CCH, 256], BF16, name=f"gath{b}_sb")
                r0 = GROUP * b
                for r in range(GROUP):
                    nc.sync.dma_start(gath[:, r, :], a2a1_out[r0 + r])
                for r in range(GROUP):
                    nc.sync.dma_start(
                        gath[64 * (r % 2) : 64 * (r % 2) + 64, 4 + r // 2, :],
                        a2a2_out[r0 + r],
                    )
                if dump:
                    nc.sync.dma_start(
                        dump_tensor(f"dgath{b}", [128, CCH, 256], BF16).ap(),
                        gath,
                    )
                gaths.append(gath)

            # ---------------- output projection + bias ----------------
            # outsb tile (b, m) = my 256-token slice of batch b, 128-token
            # half m.  Chunks 0-3 (A/B features) accumulate first so the PE
            # works while the head-C AllToAll completes; chunks 4-5 finish
            # the contraction.  The output DMA is split per tile.
            outsb = cpool.tile([128, GROUP, C], F32, name="outsb")

            def proj_phase_a(b, m):
                ppv = ps2.tile([128, 1024], F32, tag="g", name=f"pp{b}_{m}")
                for nslc in (slice(0, 512), slice(512, 768)):
                    for c in range(4):
                        nc.tensor.matmul(
                            ppv[:, nslc],
                            gaths[b][:, c, m * 128 : (m + 1) * 128],
                            wp[:, c, nslc],
                            start=(c == 0),
                            stop=False,
                        )
                return ppv

            def proj_phase_b(b, m, ppv):
                for nslc in (slice(0, 512), slice(512, 768)):
                    for c in range(4, CCH):
                        nc.tensor.matmul(
                            ppv[:, nslc],
                            gaths[b][:, c, m * 128 : (m + 1) * 128],
                            wp[:, c, nslc],
                            start=False,
                            stop=(c == CCH - 1),
                        )
                nc.vector.tensor_add(
                    out=outsb[:, 2 * b + m, :], in0=ppv[:, 0:768], in1=bias
                )
                nc.sync.dma_start(
                    out_d.ap().rearrange("(m p) f -> p m f", p=128)[
                        :, 2 * b + m, :
                    ],
                    outsb[:, 2 * b + m, :],
                )

            # Two tiles at a time (the psum pool has 3 slots): batch 0's A/B
            # contraction runs while the head-C AllToAll is in flight.
            for b in range(B):
                ppvs = [proj_phase_a(b, m) for m in range(2)]
                for m in range(2):
                    proj_phase_b(b, m, ppvs[m])

    nc.compile()
    return nc


def make_in_maps(x, w_qkv, w_proj, b_proj):
    x = np.asarray(x, np.float32)
    w_qkv = np.asarray(w_qkv, np.float32)
    w_proj = np.asarray(w_proj, np.float32)
    b_proj = np.asarray(b_proj, np.float32)

    amask_np = (np.eye(NT, dtype=np.float32) - 1.0).astype(NPBF16)
    bias_np = np.ascontiguousarray(np.tile(b_proj[None, :], (128, 1)))
    # w_proj rows permuted to the gathered feature order: chunks 0-3 hold the
    # four ranks' A/B heads ([3r | 3r+1] per chunk), chunks 4-5 the C heads.
    perm = np.zeros(C, np.int64)
    for c in range(4):
        for p in range(128):
            perm[c * 128 + p] = (3 * c + p // 64) * D + (p % 64)
    for cc in range(2):
        for p in range(128):
            perm[512 + cc * 128 + p] = (3 * (cc * 2 + p // 64) + 2) * D + (
                p % 64
            )
    wp_np = np.ascontiguousarray(w_proj[perm, :]).astype(NPBF16)

    def q(h):
        return w_qkv[:, h * D : (h + 1) * D]

    def k(h):
        return w_qkv[:, C + h * D : C + (h + 1) * D]

    def v(h):
        return w_qkv[:, 2 * C + h * D : 2 * C + (h + 1) * D]

    in_maps = []
    for core in range(NCORES):
        g, p = divmod(core, GROUP)
        hs = [HPC * p, HPC * p + 1, HPC * p + 2]
        wqk_np = np.concatenate(
            [
                q(hs[0]), q(hs[1]),           # block 0: [Q_A | Q_B]
                k(hs[0]), k(hs[1]),           # block 1: [K_A | K_B]
                q(hs[2]), q(hs[2]),           # block 2: [Q_C | Q_C]
                k(hs[2]), k(hs[2]),           # block 3: [K_C | K_C]
            ],
            axis=1,
        ).astype(NPBF16)
        wv_np = np.concatenate([v(hs[0]), v(hs[1]), v(hs[2])], axis=1).astype(
            NPBF16
        )
        xt_np = np.ascontiguousarray(x[g].T).astype(NPBF16)
        in_maps.append(
            {
                "xt": xt_np,
                "wqk": np.ascontiguousarray(wqk_np),
                "wv": np.ascontiguousarray(wv_np),
                "wp": np.ascontiguousarray(wp_np),
                "bias": bias_np,
                "amask": amask_np,
            }
        )
    return in_maps


def assemble_out(results):
    # core i returns [512, 768]: rows 0-255 = batch 0 tokens [256i, 256i+256),
    # rows 256-511 = the same token slice of batch 1.
    out = np.empty((B, N, C), np.float32)
    for core in range(NCORES):
        r = np.asarray(results[core]["out"], np.float32)
        out[0, core * 256 : (core + 1) * 256, :] = r[0:256]
        out[1, core * 256 : (core + 1) * 256, :] = r[256:512]
    return out


_CACHE = {}


def _get_nc():
    if "nc" not in _CACHE:
        _CACHE["nc"] = build_graph()
    return _CACHE["nc"]


def kernel(x, w_qkv, w_proj, b_proj):
    nc = _get_nc()
    in_maps = make_in_maps(x, w_qkv, w_proj, b_proj)
    res = run_bass_kernel_spmd(nc, in_maps, core_ids=list(range(NCORES)))
    return assemble_out(res.results)


# revision 33
# speedup vs baseline: 1.4705x; 1.4705x over previous
"""Distributed Bass kernel for nn_Attention_15247133900834.

Full-input -> full-output multi-head attention block on 8 TRN2 NeuronCores.

Sharding (hardcoded): B=2 batches x 12 heads.  Core i handles batch g=i//4
and heads [3p, 3p+1, 3p+2] with p=i%4.  Each core computes QKV + attention
for its 3 heads over ALL 2048 tokens, then an AllToAll inside each 4-core
batch group exchanges 512-token slices of the per-head attention outputs so
every core ends up with all 768 features for its token slice, and runs the
output projection for those 512 tokens.

Device-side layout choices (host prepares the shards accordingly):
- x is passed TRANSPOSED per batch (x[g].T as [768, 2048] bf16) because every
  matmul contracts over the feature axis, which must live on SBUF partitions.
- Q/K weights are packed per head-pair [wq_hA | wq_hB] so one [768,128] weight
  block yields Q^T of head A on partitions 0-63 and head B on partitions
  64-127; stage-2 S^T = K^T.T @ Q^T then runs as two concurrent row-group
  matmuls (contraction d=64 each) that together use the full 128-row PE array.
  The 3rd (solo) head duplicates its weights so the two row groups process
  low/high key halves concurrently instead.
- Attention probabilities stay transposed (P^T = [keys, q]) so the second
  attention matmul out^T = [V|1].T @ P^T needs no transposes anywhere; the
  extra ones-column of V makes the PE accumulate the softmax denominators
  as psum row 64 for free.  exp() runs on ScalarE straight out of PSUM with
  the attention scale folded into the activation's free affine.
- The task mask (first 4x4 block = eye) is applied as a tiny correction
  matmul that subtracts the masked contributions: rhs = P^T[0:4,0:4]*(eye-1).
  The denominators are computed over the UNMASKED row (mask is applied after
  softmax in the reference).
"""

import numpy as np
import ml_dtypes

import concourse.bass as bass
import concourse.mybir as mybir
import concourse.tile as tile
from concourse import bacc
from concourse.bass_utils import run_bass_kernel_spmd

BF16 = mybir.dt.bfloat16
F32 = mybir.dt.float32
NPBF16 = ml_dtypes.bfloat16

B, N, C = 2, 2048, 768
H, D = 12, 64
NT = 4              # tokens with the eye task-mask
NCORES = 8
GROUP = 4           # cores per batch group
TOKS = N // GROUP   # 512 output tokens per core
HPC = 3             # heads per core
CCH = C // 128      # 6 feature contraction chunks
KCH = N // 128      # 16 key chunks
QT = N // 512       # 4 q tiles
SCALE = D ** -0.5
EXP = mybir.ActivationFunctionType.Exp

REPLICA_GROUPS = [list(range(NCORES))]


def build_graph(dump: bool = False):
    nc = bacc.Bacc(
        "TRN2", target_bir_lowering=False, debug=False, num_devices=NCORES
    )
    dumps = {}

    def dump_tensor(name, shape, dtype):
        if not dump:
            return None
        d = nc.dram_tensor(name, shape, dtype, kind="ExternalOutput")
        dumps[name] = d
        return d
    xt_d = nc.dram_tensor("xt", [C, N], BF16, kind="ExternalInput")
    wqk_d = nc.dram_tensor("wqk", [C, 512], BF16, kind="ExternalInput")
    wv_d = nc.dram_tensor("wv", [C, HPC * D], BF16, kind="ExternalInput")
    wp_d = nc.dram_tensor("wp", [C, C], BF16, kind="ExternalInput")
    bias_d = nc.dram_tensor("bias", [128, C], F32, kind="ExternalInput")
    am_d = nc.dram_tensor("amask", [NT, NT], BF16, kind="ExternalInput")
    out_d = nc.dram_tensor("out", [TOKS, C], F32, kind="ExternalOutput")

    with tile.TileContext(nc) as tc:
        with (
            tc.tile_pool(name="const", bufs=1) as cpool,
            tc.tile_pool(name="work", bufs=3) as wpool,
            tc.tile_pool(name="pt", bufs=4) as ptpool,
            tc.tile_pool(name="ps2", bufs=3, space="PSUM") as ps2,
            tc.tile_pool(name="pso", bufs=2, space="PSUM") as pso,
            tc.tile_pool(name="dram", bufs=1, space="DRAM") as dpool,
        ):
            # ---------------- constant loads ----------------
            xt = cpool.tile([128, CCH, N], BF16, name="xt_sb")
            nc.sync.dma_start(xt, xt_d.ap().rearrange("(c p) n -> p c n", p=128))
            wqk = cpool.tile([128, CCH, 512], BF16, name="wqk_sb")
            nc.sync.dma_start(wqk, wqk_d.ap().rearrange("(c p) m -> p c m", p=128))
            wv = cpool.tile([128, CCH, HPC * D], BF16, name="wv_sb")
            nc.sync.dma_start(wv, wv_d.ap().rearrange("(c p) m -> p c m", p=128))
            wp = cpool.tile([128, CCH, C], BF16, name="wp_sb")
            nc.sync.dma_start(wp, wp_d.ap().rearrange("(c p) m -> p c m", p=128))
            bias = cpool.tile([128, C], F32, name="bias_sb")
            nc.sync.dma_start(bias, bias_d.ap())
            amask = cpool.tile([NT, NT], BF16, name="amask_sb")
            nc.sync.dma_start(amask, am_d.ap())

            # ---------------- stage 1a: Q^T / K^T blocks ----------------
            # qk block m: 0 = [Q_A; Q_B], 1 = [K_A; K_B], 2 = [Q_C; Q_C],
            # 3 = [K_C; K_C]  (head X on partitions 0-63 / 64-127)
            qk = cpool.tile([128, 4, N], BF16, name="qk_sb")

            def make_qk_block(m):
                for t in range(2):
                    pq = ps2.tile(
                        [128, 1024], F32, tag="g", name=f"pq{m}_{t}"
                    )
                    for c in range(CCH):
                        for h in range(2):
                            nc.tensor.matmul(
                                pq[:, h * 512 : (h + 1) * 512],
                                wqk[:, c, m * 128 : (m + 1) * 128],
                                xt[:, c, t * 1024 + h * 512 : t * 1024 + (h + 1) * 512],
                                start=(c == 0),
                                stop=(c == CCH - 1),
                            )
                    nc.vector.tensor_copy(
                        out=qk[:, m, t * 1024 : (t + 1) * 1024], in_=pq
                    )

            # ---------------- stage 1b: V natural + 64 ones columns -------
            # vt[:, k, h*128 : h*128+64] = V_h keys chunk k; cols h*128+64 ..
            # h*128+127 are all-ones, so stage-3's [V_h | 1..1].T @ P^T puts
            # the softmax denominators in psum rows 64-127, already broadcast
            # 64-wide for the normalization multiply.
            vt = cpool.tile([128, KCH, HPC * 128], BF16, name="vt_sb")
            ones_view = vt.rearrange("p k (h e) -> p k h e", e=128)[
                :, :, :, 64:128
            ]
            nc.vector.memset(ones_view, 1.0)

            def make_v_chunk(n_):
                pv = ps2.tile([128, 1024], F32, tag="g", name=f"pv{n_}")
                for c in range(CCH):
                    nc.tensor.matmul(
                        pv[:, 0 : HPC * D],
                        xt[:, c, n_ * 128 : (n_ + 1) * 128],
                        wv[:, c, :],
                        start=(c == 0),
                        stop=(c == CCH - 1),
                    )
                nc.vector.tensor_copy(
                    out=vt.rearrange("p k (h e) -> p k h e", e=128)[
                        :, n_, :, 0:64
                    ],
                    in_=pv[:, 0 : HPC * D].rearrange("p (h e) -> p h e", e=64),
                )

            # Emit only what the pair unit needs up front; the solo head's
            # Q/K blocks are emitted after the pair rounds so they fill PE
            # gaps while ScalarE drains the last pair exps.
            make_qk_block(0)
            make_qk_block(1)
            for n_ in range(KCH):
                make_v_chunk(n_)

            # ---------------- attention rounds ----------------
            attnAB = cpool.tile([128, N], BF16, name="attnAB_sb")
            attnC = cpool.tile([64, N], BF16, name="attnC_sb")

            # AllToAll bounce buffers (declared up front; the heads-A/B
            # exchange is launched right after the pair unit finishes so it
            # overlaps the solo head's compute).  Destination d owns tokens
            # [256d, 256d+256) of BOTH batches; my shard d = my head-features
            # for those tokens.  After the exchange, output shard j = rank
            # j's features for MY tokens: shards 0-3 = batch-0 heads,
            # shards 4-7 = batch-1 heads.
            a2a1_in = dpool.tile([NCORES, 128, 256], BF16, name="a2a1_in")
            a2a1_out = dpool.tile([NCORES, 128, 256], BF16, name="a2a1_out")
            a2a2_in = dpool.tile([NCORES, 64, 256], BF16, name="a2a2_in")
            a2a2_out = dpool.tile([NCORES, 64, 256], BF16, name="a2a2_out")

            # (kind, q block, k block, [(head col base, attn dest fn)])
            units = [
                ("pair", 0, 1),
                ("solo", 2, 3),
            ]
            for kind_, qb, kb in units:
                if kind_ == "pair":
                    heads = [(0 * 128, attnAB, 0), (1 * 128, attnAB, 64)]
                else:
                    heads = [(2 * 128, attnC, 0)]
                for t in range(QT):
                    qs = slice(t * 512, (t + 1) * 512)
                    pos = [
                        pso.tile([128, 512], F32, tag="o", name=f"o{kind_}{t}_{i}")
                        for i in range(len(heads))
                    ]
                    started = [False] * len(heads)
                    # One psum tile per key chunk: bank 0 = head A's scores
                    # (pair) / low-key chunk (solo), bank 1 = head B / high
                    # keys.  The two row-group matmuls land in the two banks
                    # and a single FD-1024 exp converts both.
                    nch = KCH if kind_ == "pair" else KCH // 2
                    for cc in range(nch):
                        g2 = ps2.tile(
                            [128, 2, 512], F32, tag="g", name=f"g{kind_}{t}_{cc}"
                        )
                        kcol = cc if kind_ == "pair" else 8 + cc
                        nc.tensor.matmul(
                            g2[:, 0, :],
                            qk[0:64, kb, cc * 128 : (cc + 1) * 128],
                            qk[0:64, qb, qs],
                            start=True,
                            stop=True,
                        )
                        nc.tensor.matmul(
                            g2[:, 1, :],
                            qk[64:128, kb, kcol * 128 : (kcol + 1) * 128],
                            qk[64:128, qb, qs],
                            start=True,
                            stop=True,
                        )
                        pt2 = ptpool.tile(
                            [128, 2, 512], BF16, tag="pt",
                            name=f"pt{kind_}{t}_{cc}",
                        )
                        nc.scalar.activation(pt2, g2, EXP, scale=SCALE)
                        if dump and kind_ == "pair" and t == 0 and cc == 0:
                            nc.sync.dma_start(
                                dump_tensor("dpt", [128, 2, 512], BF16).ap(),
                                pt2,
                            )

                        # stage 3: out^T += [V_h | 1..1].T @ P^T_chunk
                        if kind_ == "pair":
                            mms = [(0, cc, pt2[:, 0, :]), (1, cc, pt2[:, 1, :])]
                        else:
                            mms = [
                                (0, cc, pt2[:, 0, :]),
                                (0, 8 + cc, pt2[:, 1, :]),
                            ]
                        for i, kchunk, pt_ap in mms:
                            colb = heads[i][0]
                            nc.tensor.matmul(
                                pos[i],
                                vt[:, kchunk, colb : colb + 128],
                                pt_ap,
                                start=not started[i],
                                stop=(kchunk == KCH - 1),
                            )
                            started[i] = True

                        # task-mask correction (q rows 0-3 x key rows 0-3)
                        if t == 0 and cc == 0:
                            for i in range(len(heads)):
                                colb = heads[i][0]
                                anti = wpool.tile(
                                    [128, NT], BF16, tag="anti",
                                    name=f"anti{kind_}{i}",
                                )
                                nc.vector.memset(anti, 0.0)
                                nc.vector.tensor_mul(
                                    out=anti[0:NT, :],
                                    in0=pt2[0:NT, 0, 0:NT],
                                    in1=amask,
                                )
                                nc.tensor.matmul(
                                    pos[i][0:64, 0:NT],
                                    vt[:, 0, colb : colb + 64],
                                    anti,
                                    start=False,
                                    stop=False,
                                )

                    # normalization: psum rows 64-127 already hold the softmax
                    # denominators broadcast 64-wide (the ones columns of vt).
                    # Copy each psum tile to SBUF first so the psum slot frees
                    # quickly; the slow reciprocal + multiply then run off the
                    # psum-pool critical path.
                    for i, (colb, dst, drow) in enumerate(heads):
                        ocp = wpool.tile(
                            [128, 512], F32, tag="ocp", name=f"oc{kind_}{t}_{i}"
                        )
                        nc.vector.tensor_copy(out=ocp, in_=pos[i])
                        rec = wpool.tile(
                            [64, 512], F32, tag="rec", name=f"rc{kind_}{t}_{i}"
                        )
                        nc.vector.reciprocal(out=rec, in_=ocp[64:128, :])
                        nc.vector.tensor_mul(
                            out=dst[drow : drow + 64, qs],
                            in0=ocp[0:64, :],
                            in1=rec,
                        )
                        if dump and kind_ == "pair" and t == 0 and i == 0:
                            nc.sync.dma_start(
                                dump_tensor("dpos", [128, 512], F32).ap(),
                                ocp,
                            )
                            nc.sync.dma_start(
                                dump_tensor("drec", [64, 512], F32).ap(), rec
                            )

                # after the pair unit's four q-tiles, attnAB is complete:
                # launch its AllToAll so the exchange overlaps the solo
                # head's compute, and emit the solo head's Q/K production
                # (it fills PE gaps while ScalarE drains the last pair exps).
                if kind_ == "pair":
                    for d in range(NCORES):
                        nc.sync.dma_start(
                            a2a1_in[d], attnAB[:, d * 256 : (d + 1) * 256]
                        )
                    nc.gpsimd.collective_compute(
                        "AllToAll",
                        mybir.AluOpType.bypass,
                        replica_groups=REPLICA_GROUPS,
                        ins=[a2a1_in.opt()],
                        outs=[a2a1_out.opt()],
                    )
                    make_qk_block(2)
                    make_qk_block(3)

            if dump:
                nc.sync.dma_start(
                    dump_tensor("dqk", [128, 4, N], BF16).ap(), qk
                )
                nc.sync.dma_start(
                    dump_tensor("dvt", [128, KCH, HPC * 128], BF16).ap(), vt
                )

            if dump:
                nc.sync.dma_start(
                    dump_tensor("dattnAB", [128, N], BF16).ap(), attnAB
                )
                nc.sync.dma_start(
                    dump_tensor("dattnC", [64, N], BF16).ap(), attnC
                )

            # ---------------- second AllToAll: the solo head --------------
            for d in range(NCORES):
                nc.sync.dma_start(
                    a2a2_in[d], attnC[:, d * 256 : (d + 1) * 256]
                )
            nc.gpsimd.collective_compute(
                "AllToAll",
                mybir.AluOpType.bypass,
                replica_groups=REPLICA_GROUPS,
                ins=[a2a2_in.opt()],
                outs=[a2a2_out.opt()],
            )

            # gather per batch: [768 features, 256 tokens] -> [128, 6, 256].
            # PERMUTED feature order (host permutes w_proj rows to match):
            # chunks 0-3 = the four ranks' A/B heads (from a2a1, available
            # while a2a2 is still in flight), chunks 4-5 = the C heads.
            gaths = []
            for b in range(B):
                gath = cpool.tile([128, CCH, 256], BF16, name=f"gath{b}_sb")
                r0 = GROUP * b
                for r in range(GROUP):
                    nc.sync.dma_start(gath[:, r, :], a2a1_out[r0 + r])
                for r in range(GROUP):
                    nc.sync.dma_start(
                        gath[64 * (r % 2) : 64 * (r % 2) + 64, 4 + r // 2, :],
                        a2a2_out[r0 + r],
                    )
                if dump:
                    nc.sync.dma_start(
                        dump_tensor(f"dgath{b}", [128, CCH, 256], BF16).ap(),
                        gath,
                    )
                gaths.append(gath)

            # ---------------- output projection + bias ----------------
            # outsb tile (b, m) = my 256-token slice of batch b, 128-token
            # half m.  Chunks 0-3 (A/B features) accumulate first so the PE
            # works while the head-C AllToAll completes; chunks 4-5 finish
            # the contraction.  The output DMA is split per tile.
            outsb = cpool.tile([128, GROUP, C], F32, name="outsb")

            def proj_phase_a(b, m):
                ppv = ps2.tile([128, 1024], F32, tag="g", name=f"pp{b}_{m}")
                for nslc in (slice(0, 512), slice(512, 768)):
                    for c in range(4):
                        nc.tensor.matmul(
                            ppv[:, nslc],
                            gaths[b][:, c, m * 128 : (m + 1) * 128],
                            wp[:, c, nslc],
                            start=(c == 0),
                            stop=False,
                        )
                return ppv

            def proj_phase_b(b, m, ppv):
                for nslc in (slice(0, 512), slice(512, 768)):
                    for c in range(4, CCH):
                        nc.tensor.matmul(
                            ppv[:, nslc],
                            gaths[b][:, c, m * 128 : (m + 1) * 128],
                            wp[:, c, nslc],
                            start=False,
                            stop=(c == CCH - 1),
                        )
                nc.vector.tensor_add(
                    out=outsb[:, 2 * b + m, :], in0=ppv[:, 0:768], in1=bias
                )
                nc.sync.dma_start(
                    out_d.ap().rearrange("(m p) f -> p m f", p=128)[
                        :, 2 * b + m, :
                    ],
                    outsb[:, 2 * b + m, :],
                )

            # Two tiles at a time (the psum pool has 3 slots): batch 0's A/B
            # contraction runs while the head-C AllToAll is in flight.
            for b in range(B):
                ppvs = [proj_phase_a(b, m) for m in range(2)]
                for m in range(2):
                    proj_phase_b(b, m, ppvs[m])

    nc.compile()
    return nc


def make_in_maps(x, w_qkv, w_proj, b_proj):
    x = np.asarray(x, np.float32)
    w_qkv = np.asarray(w_qkv, np.float32)
    w_proj = np.asarray(w_proj, np.float32)
    b_proj = np.asarray(b_proj, np.float32)

    amask_np = (np.eye(NT, dtype=np.float32) - 1.0).astype(NPBF16)
    bias_np = np.ascontiguousarray(np.tile(b_proj[None, :], (128, 1)))
    # w_proj rows permuted to the gathered feature order: chunks 0-3 hold the
    # four ranks' A/B heads ([3r | 3r+1] per chunk), chunks 4-5 the C heads.
    perm = np.zeros(C, np.int64)
    for c in range(4):
        for p in range(128):
            perm[c * 128 + p] = (3 * c + p // 64) * D + (p % 64)
    for cc in range(2):
        for p in range(128):
            perm[512 + cc * 128 + p] = (3 * (cc * 2 + p // 64) + 2) * D + (
                p % 64
            )
    wp_np = np.ascontiguousarray(w_proj[perm, :]).astype(NPBF16)

    def q(h):
        return w_qkv[:, h * D : (h + 1) * D]

    def k(h):
        return w_qkv[:, C + h * D : C + (h + 1) * D]

    def v(h):
        return w_qkv[:, 2 * C + h * D : 2 * C + (h + 1) * D]

    in_maps = []
    for core in range(NCORES):
        g, p = divmod(core, GROUP)
        hs = [HPC * p, HPC * p + 1, HPC * p + 2]
        wqk_np = np.concatenate(
            [
                q(hs[0]), q(hs[1]),           # block 0: [Q_A | Q_B]
                k(hs[0]), k(hs[1]),           # block 1: [K_A | K_B]
                q(hs[2]), q(hs[2]),           # block 2: [Q_C | Q_C]
                k(hs[2]), k(hs[2]),           # block 3: [K_C | K_C]
            ],
            axis=1,
        ).astype(NPBF16)
        wv_np = np.concatenate([v(hs[0]), v(hs[1]), v(hs[2])], axis=1).astype(
            NPBF16
        )
        xt_np = np.ascontiguousarray(x[g].T).astype(NPBF16)
        in_maps.append(
            {
                "xt": xt_np,
                "wqk": np.ascontiguousarray(wqk_np),
                "wv": np.ascontiguousarray(wv_np),
                "wp": np.ascontiguousarray(wp_np),
                "bias": bias_np,
                "amask": amask_np,
            }
        )
    return in_maps


def assemble_out(results):
    # core i returns [512, 768]: rows 0-255 = batch 0 tokens [256i, 256i+256),
    # rows 256-511 = the same token slice of batch 1.
    out = np.empty((B, N, C), np.float32)
    for core in range(NCORES):
        r = np.asarray(results[core]["out"], np.float32)
        out[0, core * 256 : (core + 1) * 256, :] = r[0:256]
        out[1, core * 256 : (core + 1) * 256, :] = r[256:512]
    return out


_CACHE = {}


def _get_nc():
    if "nc" not in _CACHE:
        _CACHE["nc"] = build_graph()
    return _CACHE["nc"]


def kernel(x, w_qkv, w_proj, b_proj):
    nc = _get_nc()
    in_maps = make_in_maps(x, w_qkv, w_proj, b_proj)
    res = run_bass_kernel_spmd(nc, in_maps, core_ids=list(range(NCORES)))
    return assemble_out(res.results)
